# revision 1
# baseline (speedup 1.0000x reference)
"""BRITS GRU-cell recurrence on 8 Trainium2 NeuronCores.

Problem: B=8192 samples, T=256 timesteps, H=128 hidden. Data-parallel:
each core runs Bs=1024 samples through the full sequential recurrence.

Device layout per core: hidden state h lives in SBUF as [H=128 partitions,
Bs free]. All matmuls contract over H on the partition dim (fp32r = 1
cycle/row). Per-sample scalars (x_t, m_t, c_t, pred_t) are [1or2, Bs] rows.
I/O rows are staged in G-step blocks so DMAs are 32KB, not 4KB.

Per step:
  c      = tanh(clogit_prev + Wc_b)                  (ACT)
  omc    = (1-m)*c                                   (GPSIMD, into mxc row0)
  featpre= Wx (x) omc + Wx (x) mx                    (K=2 matmul; == Wx (x) x_imp)
  feat   = relu(featpre + Wx_b)                      (ACT)
  acc_rz = [Wih_r|Wih_z]@feat + [Whh_r|Whh_z]@h + [wm;b]@[m;1]  (PSUM [128,2048])
  rz     = sigmoid(acc_rz)                           (one ACT over 2048)
  i_n    = Wih_n@feat + [wm_n;b_ihn]@[m;1]           (PSUM)
  h_n    = Whh_n@h                                   (PSUM)
  rhn    = (h_n + b_hhn) * r                         (DVE scalar_tensor_tensor)
  n      = tanh(i_n + rhn)                           (DVE add + ACT)
  h      = n + z*(h-n)                               (GPSIMD sub/mul + DVE add)
  small  = [Wc_w|out_w].T @ h  -> [clogit; pred]     (M=2 matmul; DVE-copied to stage)
"""

import os
import sys
from contextlib import ExitStack

import numpy as np

for _p in ("/opt/trn_rl_repo", "/opt/pypackages"):
    if _p not in sys.path and os.path.isdir(_p):
        sys.path.append(_p)

import concourse.bass as bass
import concourse.bacc as bacc
import concourse.tile as tile
from concourse import mybir
from concourse.bass_utils import run_bass_kernel_spmd

B, T, H = 8192, 256, 128
NCORES = 8
BS = B // NCORES  # 1024 samples per core
NT = 2            # column tiles of 512
TN = BS // NT     # 512
G = 4             # steps per I/O block
F32 = mybir.dt.float32
F32R = mybir.dt.float32r


def r(ap):
    return ap.bitcast(F32R)


def build_program(t_steps=T, bs=BS):
    assert t_steps % G == 0
    nc = bacc.Bacc("TRN2", target_bir_lowering=False, debug=False)
    gbs = G * bs

    xm = nc.dram_tensor("xm", [t_steps, 3, bs], F32R, kind="ExternalInput").ap()
    wihT = nc.dram_tensor("wihT", [H, 3 * H], F32R, kind="ExternalInput").ap()
    whhT = nc.dram_tensor("whhT", [H, 3 * H], F32R, kind="ExternalInput").ap()
    wmb = nc.dram_tensor("wmb", [2, 3 * H], F32R, kind="ExternalInput").ap()
    wxw = nc.dram_tensor("wxw", [2, H], F32R, kind="ExternalInput").ap()
    wsmall = nc.dram_tensor("wsmall", [H, 2], F32R, kind="ExternalInput").ap()
    biases = nc.dram_tensor("biases", [H, 3], F32, kind="ExternalInput").ap()
    hz = nc.dram_tensor("hz", [H, bs], F32R, kind="ExternalInput").ap()
    mone = nc.dram_tensor("mone", [1, gbs], F32R, kind="ExternalInput").ap()
    opc = nc.dram_tensor("opc", [t_steps, 2, bs], F32, kind="ExternalOutput").ap()

    AF = mybir.ActivationFunctionType
    OP = mybir.AluOpType

    with tile.TileContext(nc) as tc, ExitStack() as ctx:
        const = ctx.enter_context(tc.tile_pool(name="const", bufs=1))
        work = ctx.enter_context(tc.tile_pool(name="work", bufs=2))
        ps_rz = ctx.enter_context(tc.tile_pool(name="ps_rz", bufs=1, space="PSUM"))
        ps_a = ctx.enter_context(tc.tile_pool(name="ps_a", bufs=1, space="PSUM"))
        ps_b = ctx.enter_context(tc.tile_pool(name="ps_b", bufs=1, space="PSUM"))

        # --- constants / persistent state ---
        w_ih = const.tile([H, 3 * H], F32R)
        nc.sync.dma_start(w_ih[:], wihT[:])
        w_hh = const.tile([H, 3 * H], F32R)
        nc.sync.dma_start(w_hh[:], whhT[:])
        w_mb = const.tile([2, 3 * H], F32R)
        nc.sync.dma_start(w_mb[:], wmb[:])
        w_xw = const.tile([2, H], F32R)
        nc.sync.dma_start(w_xw[:], wxw[:])
        w_sm = const.tile([H, 2], F32R)
        nc.sync.dma_start(w_sm[:], wsmall[:])
        bia = const.tile([H, 3], F32)
        nc.sync.dma_start(bia[:], biases[:])

        h = const.tile([H, bs], F32R)
        nc.sync.dma_start(h[:], hz[:])
        zrow = const.tile([1, bs], F32)
        nc.vector.memset(zrow[:], 0.0)

        # double-buffered per-block staging (persistent tiles)
        mrow_ab, mxc_ab, om_ab, cp_ab = [], [], [], []
        for i_ in range(2):
            mt = const.tile([2, gbs], F32R, tag=f"mrow{i_}")
            nc.sync.dma_start(mt[1:2, :], mone[:])   # ones row for [m;1] rhs
            mrow_ab.append(mt)
            mxc_ab.append(const.tile([2, gbs], F32R, tag=f"mxc{i_}", name=f"mxc{i_}"))
            om_ab.append(const.tile([1, gbs], F32, tag=f"om{i_}", name=f"om{i_}"))
            cp_ab.append(const.tile([2, gbs], F32, tag=f"cp{i_}", name=f"cp{i_}"))

        tc.strict_bb_all_engine_barrier()

        b_hhn = bia[:, 0:1]
        b_wx = bia[:, 1:2]
        b_wc = bia[0:1, 2:3]

        h_b, feat_p, rz_p, rhn_p, npre_p, n_p, tmp_p = [], [], [], [], [], [], []
        ps_acc, ps_fp, ps_hn = [], [], []
        for b_ in range(NT):
            hb = const.tile([H, TN], F32R, tag=f"h{b_}", name=f"h{b_}")
            nc.sync.dma_start(hb[:], hz[:, b_ * TN:(b_ + 1) * TN])
            h_b.append(hb)
            ps_acc.append(ctx.enter_context(
                tc.tile_pool(name=f"psacc{b_}", bufs=1, space="PSUM")))
            ps_fp.append(ctx.enter_context(
                tc.tile_pool(name=f"psfp{b_}", bufs=1, space="PSUM")))
            ps_hn.append(ctx.enter_context(
                tc.tile_pool(name=f"pshn{b_}", bufs=1, space="PSUM")))

        prev_small = [None, None]

        for t in range(t_steps):
            g = t % G
            blk = (t // G) % 2
            off = g * bs

            if g == 0:
                t0 = t
                nc.sync.dma_start(mrow_ab[blk][0:1, :], xm[t0:t0 + G, 0, :])
                nc.sync.dma_start(mxc_ab[blk][1:2, :], xm[t0:t0 + G, 1, :])
                nc.sync.dma_start(om_ab[blk][0:1, :].bitcast(F32R),
                                  xm[t0:t0 + G, 2, :])
            mrow = mrow_ab[blk]
            mxc = mxc_ab[blk]

            for b in range(NT):
                h = h_b[b]
                sob = slice(off + b * TN, off + (b + 1) * TN)

                # -- c = tanh(clogit + Wc_b) into mxc row0; clogit from prev
                #    step's small-matmul PSUM --
                if prev_small[b] is None:
                    cl_src = zrow[0:1, 0:TN]
                else:
                    cl_src = prev_small[b][0:1, :]
                nc.scalar.activation(mxc[0:1, sob], cl_src, AF.Tanh, bias=b_wc)

                # -- omc = (1-m)*c in place (gpsimd) --
                nc.gpsimd.tensor_mul(mxc[0:1, sob], om_ab[blk][0:1, sob],
                                     mxc[0:1, sob].bitcast(F32))

                # -- featpre = Wx (x) [omc; mx] --
                featpre = ps_fp[b].tile([H, TN], F32, tag=f"infp{b}",
                                        name=f"fp{b}_{t}")
                nc.tensor.matmul(featpre[:], w_xw[:, :], mxc[0:2, sob],
                                 start=True, stop=True)
                feat = work.tile([H, TN], F32R, tag=f"feat{b}", name=f"ft{b}_{t}")
                if b == 0:
                    nc.scalar.activation(feat[:], featpre[:], AF.Relu, bias=b_wx)
                else:
                    nc.vector.tensor_scalar(feat[:], featpre[:], b_wx, 0.0,
                                            OP.add, OP.max)

                # -- gate matmuls --
                acc = ps_acc[b].tile([H, 2 * TN], F32, tag=f"acc{b}",
                                     name=f"acc{b}_{t}")
                i_n = ps_fp[b].tile([H, TN], F32, tag=f"infp{b}",
                                    name=f"in{b}_{t}")
                h_n = ps_hn[b].tile([H, TN], F32, tag=f"hn{b}", name=f"hn{b}_{t}")
                sr, sz_ = slice(0, TN), slice(TN, 2 * TN)
                nc.tensor.matmul(acc[:, sr], w_ih[:, 0:H], feat[:],
                                 start=True, stop=False)
                nc.tensor.matmul(acc[:, sr], w_hh[:, 0:H], h[:],
                                 start=False, stop=False)
                nc.tensor.matmul(acc[:, sr], w_mb[:, 0:H], mrow[:, sob],
                                 start=False, stop=True)
                nc.tensor.matmul(acc[:, sz_], w_ih[:, H:2 * H], feat[:],
                                 start=True, stop=False)
                nc.tensor.matmul(acc[:, sz_], w_hh[:, H:2 * H], h[:],
                                 start=False, stop=False)
                nc.tensor.matmul(acc[:, sz_], w_mb[:, H:2 * H], mrow[:, sob],
                                 start=False, stop=True)
                nc.tensor.matmul(i_n[:], w_ih[:, 2 * H:], feat[:],
                                 start=True, stop=False)
                nc.tensor.matmul(i_n[:], w_mb[:, 2 * H:], mrow[:, sob],
                                 start=False, stop=True)
                nc.tensor.matmul(h_n[:], w_hh[:, 2 * H:], h[:],
                                 start=True, stop=True)

                # -- gates --
                rz = work.tile([H, 2 * TN], F32, tag=f"rz{b}", name=f"rz{b}_{t}")
                nc.scalar.activation(rz[:], acc[:], AF.Sigmoid)
                rhn = work.tile([H, TN], F32, tag=f"rhn{b}", bufs=1,
                                name=f"rh{b}_{t}")
                nc.vector.scalar_tensor_tensor(rhn[:], h_n[:], b_hhn,
                                               rz[:, sr], OP.add, OP.mult)
                npre = work.tile([H, TN], F32, tag=f"npre{b}", bufs=1,
                                 name=f"np{b}_{t}")
                nc.vector.tensor_add(npre[:], i_n[:], rhn[:])
                n_sb = work.tile([H, TN], F32, tag=f"n{b}", name=f"n{b}_{t}")
                nc.scalar.activation(n_sb[:], npre[:], AF.Tanh)

                # -- h = n + z*(h-n) --
                tmp = work.tile([H, TN], F32, tag=f"tmp{b}", bufs=1,
                                name=f"tp{b}_{t}")
                nc.gpsimd.tensor_sub(tmp[:], h[:].bitcast(F32), n_sb[:])
                nc.gpsimd.tensor_mul(tmp[:], tmp[:], rz[:, sz_])
                if b == 0:
                    nc.gpsimd.tensor_add(h[:], n_sb[:], tmp[:])
                else:
                    nc.vector.tensor_add(h[:], n_sb[:], tmp[:])

                # -- small matmul: [clogit; pred]; kept in PSUM for next step --
                small = ps_acc[b].tile([2, TN], F32, tag=f"acc{b}",
                                       name=f"sm{b}_{t}")
                nc.tensor.matmul(small[:], w_sm[:, :], h[:], start=True, stop=True)
                prev_small[b] = small
                # lazy copy to output stage
                nc.vector.tensor_copy(cp_ab[blk][0:2, sob], small[:])

            if g == G - 1:
                t0 = t - G + 1
                nc.sync.dma_start(opc[t0:t0 + G, 0, :], cp_ab[blk][0:1, :])
                nc.sync.dma_start(opc[t0:t0 + G, 1, :], cp_ab[blk][1:2, :])

    nc.compile()
    return nc


def make_in_maps(x_seq, m_seq, Wc_w, Wc_b, Wx_w, Wx_b, W_ih, W_hh, b_ih, b_hh,
                 out_w, out_b, t_steps=T, bs=BS, ncores=NCORES):
    f = np.float32
    wihT = np.ascontiguousarray(W_ih[:, :H].T, dtype=f)          # [128, 384]
    whhT = np.ascontiguousarray(W_hh.T, dtype=f)                 # [128, 384]
    wmb = np.empty((2, 3 * H), dtype=f)
    wmb[0] = W_ih[:, H]
    wmb[1, 0:H] = b_ih[0:H] + b_hh[0:H]
    wmb[1, H:2 * H] = b_ih[H:2 * H] + b_hh[H:2 * H]
    wmb[1, 2 * H:] = b_ih[2 * H:]
    wxw = np.ascontiguousarray(
        np.stack([Wx_w[:, 0], Wx_w[:, 0]], axis=0), dtype=f)     # [2, 128]
    wsmall = np.stack([Wc_w[0], out_w[0]], axis=1).astype(f)     # [128, 2]
    biases = np.zeros((H, 3), dtype=f)
    biases[:, 0] = b_hh[2 * H:]
    biases[:, 1] = Wx_b
    biases[0, 2] = Wc_b[0]

    xT = np.ascontiguousarray(x_seq.T, dtype=f)  # [T, B]
    mT = np.ascontiguousarray(m_seq.T, dtype=f)

    in_maps = []
    for i in range(ncores):
        sl = slice(i * bs, (i + 1) * bs)
        xmc = np.empty((t_steps, 3, bs), dtype=f)
        xmc[:, 0, :] = mT[:t_steps, sl]
        xmc[:, 1, :] = mT[:t_steps, sl] * xT[:t_steps, sl]
        xmc[:, 2, :] = 1.0 - mT[:t_steps, sl]
        in_maps.append({
            "xm": xmc, "wihT": wihT, "whhT": whhT, "wmb": wmb, "wxw": wxw,
            "wsmall": wsmall, "biases": biases,
            "hz": np.zeros((H, bs), dtype=f),
            "mone": np.ones((1, G * bs), dtype=f),
        })
    return in_maps


_CACHE = {}


def kernel(x_seq, m_seq, Wc_w, Wc_b, Wx_w, Wx_b, W_ih, W_hh, b_ih, b_hh,
           out_w, out_b):
    Wc_b = np.asarray(Wc_b)
    x_seq = np.asarray(x_seq, dtype=np.float32)
    m_seq = np.asarray(m_seq, dtype=np.float32)
    if "nc" not in _CACHE:
        _CACHE["nc"] = build_program()
    nc = _CACHE["nc"]
    in_maps = make_in_maps(x_seq, m_seq, np.asarray(Wc_w), np.asarray(Wc_b),
                           np.asarray(Wx_w), np.asarray(Wx_b), np.asarray(W_ih),
                           np.asarray(W_hh), np.asarray(b_ih), np.asarray(b_hh),
                           np.asarray(out_w), np.asarray(out_b))
    res = run_bass_kernel_spmd(nc, in_maps, list(range(NCORES)))
    preds = np.empty((B, T), dtype=np.float32)
    comps = np.empty((B, T), dtype=np.float32)
    for i in range(NCORES):
        o = res.results[i]["opc"]  # [T, 2, BS]
        sl = slice(i * BS, (i + 1) * BS)
        cl = o[:, 0, :].T  # clogit produced at step t = logit of c_{t+1}
        comps[sl, 1:] = np.tanh(cl[:, :-1] + np.float32(Wc_b[0]))
        comps[sl, 0] = np.tanh(np.float32(Wc_b[0]))
        preds[sl, :] = o[:, 1, :].T + np.float32(out_b[0])
    return preds, comps

